# revision 39
# baseline (speedup 1.0000x reference)
"""Bass/Tile TRN2 kernel for nn_MultiHeadAttention_9277129359942.

B=2, T=S=2048, D=1024, H=16 heads, head_dim=64, fp32 I/O.

Sharding (8 cores): data-parallel over batch (2) x tensor-parallel over
head groups (4 heads / core, 256 out dims).  Each core computes the
attention for its 4 heads and a partial output projection; the host sums
the 4 bf16 partials per batch and adds the (linear) bo and bv terms
exactly: out = sum_g partial_g + bo + bv @ Wo.T.

v6 design (~221-228us measured vs v3 baseline ~315us; rel err 1.396e-2):
  - ctx matmuls col-tile-packed: head A -> psum partitions 0-63
    (col groups 0,1), head B -> 64-127 (groups 2,3), concurrent on the
    PE array.  The ones-column denominator trick is replaced by four
    M=1 denominator matmuls per iteration, col-tiled to psum partitions
    0/32/64/96 of one shared bank - all four run concurrently, so the
    denominators cost ~1 extra matmul-stream instead of widening ctx
    to M=65 (which would forfeit the 2x col packing).
  - exp split by t-chunk instead of by head: ACT exact exp on
    [scoresA-c0 | scoresB-c0] (one [128,1024] instr), DVE fast-exp
    (Schraudolph int16) on [A-c1 | B-c1].  Steady-state iteration is
    ~1.74us, bound by the DVE fast-exp (fp32-psum-read limited,
    ~1.22us) -> scores WAR cycle; psum (8 banks: sc 4 + ctx 2 + dn 1
    + scratch 1) cannot double-buffer the score tiles.
  - the ACT engine doubles as the scalar HWDGE queue: ACT compute ops
    emitted after bulk DMAs serialize behind the whole outstanding
    input stream.  Hence the exp-table preload is emitted before the
    x/w DMA block and ALL projection-phase evictions run on the DVE
    (pass 2 of the q/k projection is group-outer with inline evicts;
    the v projection is k-outer chasing the xv DMAs, with the psum
    `start` flag only on the even-s matmul of each shared bank - the
    reset clears the full bank row of every written partition).
  - blocks stream back-to-back on the PE; per-block normalize:
    denominators drain first, dn evicts, 4 compact DMAs -> [128,16] ->
    one reciprocal_approx_fast, then a log2 DMA broadcast chain +
    GpSimd multiply (blocks 0-2, fully off the critical path).  The
    last block casts the recip to bf16 and defers its broadcast (two
    pairs of concurrent K=1 PE matmuls into popsum) until after the
    th=0 out-proj tiles, so the tail has no DMA-latency stall; an
    out-proj bridge tile at the last block boundary keeps HAM (the PE
    clock throttle) at full clock through the transition.
  - biases are host-pre-transposed to [128,2] (a rearranged 256-row
    DRAM DMA was scheduled at the queue tail and stalled every
    consumer ~40us).
"""

import os
import sys

import numpy as np

for _p in ("/opt/trn_rl_repo",):
    if os.path.isdir(_p) and _p not in sys.path:
        sys.path.append(_p)

import ml_dtypes

import concourse.bass as bass
import concourse.mybir as mybir
import concourse.tile as tile
from concourse import bacc
from concourse.bass_utils import run_bass_kernel_spmd

F32 = mybir.dt.float32
BF16 = mybir.dt.bfloat16
I16 = mybir.dt.int16
AF = mybir.ActivationFunctionType
ALU = mybir.AluOpType
BF16_NP = ml_dtypes.bfloat16

D = 1024          # model dim
T = 2048          # query length
S = 2048          # key length
P = 128           # partitions
KT = D // P       # 8 contraction tiles
TT = T // P       # 16 row tiles
ST = S // P       # 16 key tiles
HL = 4            # local heads per core
HD = 64           # head dim
OUTL = HL * HD    # 256 local out dims
N_CORES = 8

# fast-exp constants: exp(x*0.125) ~= bf16(bitcast(int16(x*EA + EC)))
EA = float(0.125 * 128.0 / np.log(2.0))
EC = float(127 * 128 - 7.5)
I32 = mybir.dt.int32
RMAGIC = 0x7EF311C3   # int-trick reciprocal seed constant


def build_program():
    """Build + compile the SPMD program (same on all 8 cores)."""
    nc = bacc.Bacc(
        "TRN2", target_bir_lowering=False, debug=False, enable_asserts=True,
        num_devices=N_CORES,
    )

    xq_d = nc.dram_tensor("xq", [D, T], BF16, kind="ExternalInput")
    xk_d = nc.dram_tensor("xk", [D, S], BF16, kind="ExternalInput")
    xv_d = nc.dram_tensor("xv", [D, S], BF16, kind="ExternalInput")
    wq_d = nc.dram_tensor("wq", [D, OUTL], BF16, kind="ExternalInput")
    wk_d = nc.dram_tensor("wk", [D, OUTL], BF16, kind="ExternalInput")
    wv_d = nc.dram_tensor("wv", [D, OUTL], BF16, kind="ExternalInput")
    wo_d = nc.dram_tensor("wo", [OUTL, D], BF16, kind="ExternalInput")
    bq_d = nc.dram_tensor("bq", [P, 2], F32, kind="ExternalInput")
    bk_d = nc.dram_tensor("bk", [P, 2], F32, kind="ExternalInput")
    out_d = nc.dram_tensor("out", [T, D], BF16, kind="ExternalOutput")
    wsink_d = nc.dram_tensor("warm_sink", [1, 8], F32, kind="ExternalOutput")
    dbg = {}
    if os.environ.get("BASS_MHA_DEBUG"):
        dbg["v"] = nc.dram_tensor("dbg_v", [P, ST * HL * HD], BF16,
                                  kind="ExternalOutput")
        dbg["dn"] = nc.dram_tensor("dbg_dn", [P, 512], F32,
                                   kind="ExternalOutput")
        dbg["rc"] = nc.dram_tensor("dbg_rc", [P, 16], F32,
                                   kind="ExternalOutput")
        dbg["ctxT"] = nc.dram_tensor("dbg_ctxT", [P, 1024], BF16,
                                     kind="ExternalOutput")
        dbg["qT"] = nc.dram_tensor("dbg_qT", [P, T], BF16,
                                   kind="ExternalOutput")
        dbg["kT"] = nc.dram_tensor("dbg_kT", [P, S], BF16,
                                   kind="ExternalOutput")

    with tile.TileContext(nc) as tc:
        _build(nc, tc, xq_d, xk_d, xv_d, wq_d, wk_d, wv_d, wo_d,
               bq_d, bk_d, out_d, wsink_d, dbg)
    nc.compile()
    return nc


def _build(nc, tc, xq_d, xk_d, xv_d, wq_d, wk_d, wv_d, wo_d,
           bq_d, bk_d, out_d, wsink_d, dbg={}):
    from contextlib import ExitStack

    stack = ExitStack()
    with stack:
        consts = stack.enter_context(tc.tile_pool(name="consts", bufs=1))
        wpool = stack.enter_context(tc.tile_pool(name="wpool", bufs=1))
        acts = stack.enter_context(tc.tile_pool(name="acts", bufs=1))

        bq_sb = consts.tile([P, 2], F32, name="bq", tag="bq")
        bk_sb = consts.tile([P, 2], F32, name="bk", tag="bk")
        wsnk = consts.tile([1, 8], F32, name="wsnk", tag="wsnk")
        ones_sb = consts.tile([P, 1], BF16, name="ones", tag="ones")
        onesrow = consts.tile([P, HD], BF16, name="onesrow", tag="onesrow")

        wq_sb = [wpool.tile([P, OUTL], BF16, name=f"wq{k}", tag=f"wq{k}")
                 for k in range(KT)]
        wk_sb = [wpool.tile([P, OUTL], BF16, name=f"wk{k}", tag=f"wk{k}")
                 for k in range(KT)]
        wv_sb = [wpool.tile([P, OUTL], BF16, name=f"wv{k}", tag=f"wv{k}")
                 for k in range(KT)]
        wo_sb = [wpool.tile([P, D], BF16, name=f"wo{k}", tag=f"wo{k}")
                 for k in range(2)]

        qT = [acts.tile([P, T], BF16, name=f"qT{m}", tag=f"qT{m}")
              for m in range(2)]
        kT = [acts.tile([P, S], BF16, name=f"kT{m}", tag=f"kT{m}")
              for m in range(2)]
        v_sb = acts.tile([P, ST * HL * HD], BF16, name="vsb", tag="vsb")
        ctxT = [[acts.tile([P, 1024], BF16, name=f"ctxT{g}{th}",
                           tag=f"ctxT{g}{th}") for th in range(2)]
                for g in range(2)]

        xpool_cm = tc.tile_pool(name="xpool", bufs=1)
        xpool = xpool_cm.__enter__()
        xq_sb = [xpool.tile([P, T], BF16, name=f"xq{k}", tag=f"xq{k}")
                 for k in range(KT)]
        xk_sb = [xpool.tile([P, S], BF16, name=f"xk{k}", tag=f"xk{k}")
                 for k in range(KT)]
        xv_sb = [xpool.tile([P, S], BF16, name=f"xv{k}", tag=f"xv{k}")
                 for k in range(KT)]

        # Full 128-partition DMAs (all 16 SDMA engines per transfer),
        # alternating sync/scalar queues; q/k first, v later, wo last.
        nc.sync.dma_start(wq_sb[0][:], wq_d[0:P, :])
        nc.scalar.dma_start(wk_sb[0][:], wk_d[0:P, :])
        nc.sync.dma_start(bq_sb[:], bq_d[:, :])
        nc.scalar.dma_start(bk_sb[:], bk_d[:, :])
        nc.vector.memset(ones_sb[:], 1.0)
        nc.vector.memset(onesrow[:], 1.0)
        # ACT exp-table preload, emitted before the bulk DMA stream so the
        # in-order ACT queue isn't pinned behind the full input stream
        nc.scalar.activation(wsnk[0:1, 0:2], bq_sb[0:1, 0:2], AF.Exp)
        nc.sync.dma_start(xq_sb[0][:], xq_d[0:P, :])
        nc.scalar.dma_start(xk_sb[0][:], xk_d[0:P, :])
        for k in range(1, KT):
            nc.sync.dma_start(wq_sb[k][:], wq_d[k * P:(k + 1) * P, :])
            nc.scalar.dma_start(wk_sb[k][:], wk_d[k * P:(k + 1) * P, :])
            nc.sync.dma_start(xq_sb[k][:], xq_d[k * P:(k + 1) * P, :])
            nc.scalar.dma_start(xk_sb[k][:], xk_d[k * P:(k + 1) * P, :])
        for k in range(KT):
            eng = nc.sync if k % 2 == 0 else nc.scalar
            eng.dma_start(wv_sb[k][:], wv_d[k * P:(k + 1) * P, :])
            eng2 = nc.scalar if k % 2 == 0 else nc.sync
            eng2.dma_start(xv_sb[k][:], xv_d[k * P:(k + 1) * P, :])
        for k in range(2):
            eng = nc.sync if k % 2 == 0 else nc.scalar
            eng.dma_start(wo_sb[k][:], wo_d[k * P:(k + 1) * P, :])

        # ---- q/k projections: k-outer (chases the x DMAs) over two
        # t-half passes; all four (m, q/k) psum groups live at once -----
        with tc.tile_pool(name="qkpsum", bufs=1, space="PSUM") as qkpsum:
            # no warmup burst: the first ~3.4us of real projection matmuls
            # run at the cold half-clock either way, which costs less than
            # a dedicated garbage burst delays them
            nc.vector.tensor_copy(wsnk[0:1, 2:4], bq_sb[0:1, 0:2])
            nc.sync.dma_start(wsink_d[:, :], wsnk[:])

            groups = [(m, w_sb, x_sb, b_sb, o_sb)
                      for m in range(2)
                      for w_sb, x_sb, b_sb, o_sb in
                      ((wq_sb, xq_sb, bq_sb, qT),
                       (wk_sb, xk_sb, bk_sb, kT))]
            for tg in range(2):
                t_lo = tg * 1024
                ps_g = [qkpsum.tile([P, 1024], F32, name=f"pq{gi}{tg}",
                                    tag=f"pq{gi}")
                        for gi in range(4)]
                if tg == 0:
                    # k-outer: chases the streaming x DMAs
                    for k in range(KT):
                        for gi, (m, w_sb, x_sb, b_sb, o_sb) in \
                                enumerate(groups):
                            for c in range(2):
                                cs = slice(c * 512, (c + 1) * 512)
                                xs = slice(t_lo + c * 512,
                                           t_lo + (c + 1) * 512)
                                nc.tensor.matmul(
                                    ps_g[gi][:, cs],
                                    w_sb[k][:, m * P:(m + 1) * P],
                                    x_sb[k][:, xs],
                                    start=(k == 0), stop=(k == KT - 1))
                else:
                    # group-outer: inputs resident; evict each group as it
                    # completes so the next phase's psum frees early
                    for gi, (m, w_sb, x_sb, b_sb, o_sb) in \
                            enumerate(groups):
                        for k in range(KT):
                            for c in range(2):
                                cs = slice(c * 512, (c + 1) * 512)
                                xs = slice(t_lo + c * 512,
                                           t_lo + (c + 1) * 512)
                                nc.tensor.matmul(
                                    ps_g[gi][:, cs],
                                    w_sb[k][:, m * P:(m + 1) * P],
                                    x_sb[k][:, xs],
                                    start=(k == 0), stop=(k == KT - 1))
                        eng = nc.vector if gi % 2 == 0 else nc.scalar
                        if gi % 2 == 0:
                            nc.vector.tensor_scalar(
                                o_sb[m][:, t_lo:t_lo + 1024], ps_g[gi][:],
                                b_sb[:, m:m + 1], None, op0=ALU.add)
                        else:
                            nc.scalar.activation(
                                o_sb[m][:, t_lo:t_lo + 1024], ps_g[gi][:],
                                AF.Identity, bias=b_sb[:, m:m + 1])
                if tg == 0:
                    for gi, (m, w_sb, x_sb, b_sb, o_sb) in \
                            enumerate(groups):
                        # DVE, not ACT: the ACT engine is also the scalar
                        # HWDGE queue; a compute op there serializes behind
                        # the outstanding input-DMA stream.
                        nc.vector.tensor_scalar(
                            o_sb[m][:, t_lo:t_lo + 1024], ps_g[gi][:],
                            b_sb[:, m:m + 1], None, op0=ALU.add)

        # ---- v projection (no bias: bv is applied on the host).  k-outer
        # so the matmuls chase the xv DMAs; all 16 s-accumulators live in
        # psum (8 banks, 2 s-outputs per [128,512] tile) -----------------
        with tc.tile_pool(name="vpsum", bufs=1, space="PSUM") as vpsum:
            pvs = [vpsum.tile([P, 2 * OUTL], F32, name=f"pv{j}", tag=f"pv{j}")
                   for j in range(ST // 2)]
            # psum `start` resets the FULL bank row of every partition the
            # matmul writes, so only the even-s matmul of each shared bank
            # may carry start=True; the odd-s region accumulates onto the
            # columns that reset already zeroed.
            for k in range(KT):
                for s in range(ST):
                    dst = pvs[s // 2][:, (s % 2) * OUTL:(s % 2 + 1) * OUTL]
                    nc.tensor.matmul(
                        dst, xv_sb[k][:, s * P:(s + 1) * P], wv_sb[k][:],
                        start=(k == 0 and s % 2 == 0), stop=(k == KT - 1),
                        skip_group_check=True)
                    if k == KT - 1 and s % 2 == 1:
                        j = s // 2
                        if j % 2 == 0:
                            nc.vector.tensor_copy(
                                v_sb[:, j * 2 * OUTL:(j + 1) * 2 * OUTL],
                                pvs[j][:])
                        else:
                            nc.scalar.copy(
                                v_sb[:, j * 2 * OUTL:(j + 1) * 2 * OUTL],
                                pvs[j][:])
        xpool_cm.__exit__(None, None, None)
        if dbg:
            nc.sync.dma_start(dbg["v"][:, :], v_sb[:])
            nc.sync.dma_start(dbg["qT"][:, :], qT[0][:])
            nc.sync.dma_start(dbg["kT"][:, :], kT[0][:])

        # ---- attention: 4 blocks of (head pair p, t-half th), streamed
        # back-to-back on the PE -----------------------------------------
        with tc.tile_pool(name="scpsum", bufs=1, space="PSUM") as scpsum, \
             tc.tile_pool(name="ctxpsum", bufs=1, space="PSUM") as ctxpsum, \
             tc.tile_pool(name="dnpsum", bufs=1, space="PSUM") as dnpsum, \
             tc.tile_pool(name="stgpool", bufs=2) as stgpool, \
             tc.tile_pool(name="nrmpool", bufs=2) as nrmpool, \
             tc.tile_pool(name="rbpool", bufs=2) as rbpool, \
             tc.tile_pool(name="epool", bufs=2) as epool:

            rcb_l = acts.tile([P, 16], BF16, name="rcbl", tag="rcbl")
            stg_l = None
            flt_l = acts.tile([P, 1024], BF16, name="fltl", tag="fltl")
            # block order: th=0 blocks first so the th=0 out-proj tiles
            # are unblocked long before the tail
            blocks = [(0, 0), (1, 0), (0, 1), (1, 1)]
            obr_t = [acts.tile([P, D], BF16, name=f"obr{t}", tag=f"obr{t}")
                     for t in range(TT // 2)]

            def emit_bridge_chunk(ci):
                t, n = divmod(ci, 2)
                ts_ = slice(t * P, (t + 1) * P)
                ns = slice(n * 512, (n + 1) * 512)
                bpo = dnpsum.tile([P, 512], F32, name="bpo", tag="bpo")
                for g in range(2):
                    nc.tensor.matmul(bpo[:], ctxT[g][0][:, ts_],
                                     wo_sb[g][:, ns],
                                     start=(g == 0), stop=(g == 1))
                if n == 0:
                    nc.scalar.copy(obr_t[t][:, ns], bpo[:])
                else:
                    nc.vector.tensor_copy(obr_t[t][:, ns], bpo[:])
                    eng = nc.sync if t % 2 == 0 else nc.scalar
                    eng.dma_start(out_d[t * P:(t + 1) * P, :], obr_t[t][:])

            for bi, (p, th) in enumerate(blocks):
                t0 = th * 1024
                last = bi == len(blocks) - 1
                hA = 2 * p
                hB = 2 * p + 1
                if last:
                    # boundary bridge: out-proj tile t=0 keeps the PE
                    # dense across this block boundary (HAM stays at
                    # full clock)
                    emit_bridge_chunk(0)
                    emit_bridge_chunk(1)
                ctxAB = ctxpsum.tile([P, 1024], F32, name="ctxAB",
                                     tag="ctxAB")
                dn = dnpsum.tile([P, 512], F32, name="dn", tag="dn")

                def vsl(h, s):
                    return slice(s * OUTL + h * HD, s * OUTL + (h + 1) * HD)

                prev = None
                prev2 = None
                sc23_t = None

                def emit_dn(pe0x, pe1bx, st_, so_):
                    nc.tensor.matmul(dn[0:1, :], ones_sb[:],
                                     pe0x[:, 0:512], start=st_, stop=so_,
                                     tile_position=(0, 0))
                    nc.tensor.matmul(dn[32:33, :], ones_sb[:],
                                     pe1bx[:, 0:512], start=st_, stop=so_,
                                     tile_position=(0, 32))
                    nc.tensor.matmul(dn[64:65, :], ones_sb[:],
                                     pe0x[:, 512:1024], start=st_, stop=so_,
                                     tile_position=(0, 64))
                    nc.tensor.matmul(dn[96:97, :], ones_sb[:],
                                     pe1bx[:, 512:1024],
                                     start=st_, stop=so_,
                                     tile_position=(0, 96))

                def emit_ctx(spx, pe0x, pe1bx, st_, so_):
                    vA = v_sb[:, vsl(hA, spx)]
                    vB = v_sb[:, vsl(hB, spx)]
                    nc.tensor.matmul(ctxAB[0:HD, 0:512], vA,
                                     pe0x[:, 0:512], start=st_, stop=so_)
                    nc.tensor.matmul(ctxAB[HD:P, 0:512], vB,
                                     pe0x[:, 512:1024], start=st_, stop=so_)
                    nc.tensor.matmul(ctxAB[0:HD, 512:1024], vA,
                                     pe1bx[:, 0:512], start=st_, stop=so_)
                    nc.tensor.matmul(ctxAB[HD:P, 512:1024], vB,
                                     pe1bx[:, 512:1024],
                                     start=st_, stop=so_)

                for s in range(ST):
                    ss = slice(s * P, (s + 1) * P)
                    sc01 = scpsum.tile([P, 1024], F32, name="sc01",
                                       tag="sc01")
                    sc23 = scpsum.tile([P, 1024], F32, name="sc23",
                                       tag="sc23")
                    sc23_t = sc23
                    nc.tensor.matmul(sc01[:, 0:512], kT[p][0:HD, ss],
                                     qT[p][0:HD, t0:t0 + 512],
                                     start=True, stop=True)
                    nc.tensor.matmul(sc01[:, 512:1024], kT[p][HD:P, ss],
                                     qT[p][HD:P, t0:t0 + 512],
                                     start=True, stop=True)
                    e0 = epool.tile([P, 1024], BF16, name="e0", tag="e0")
                    e1 = epool.tile([P, 1024], I16, name="e1", tag="e1")
                    nc.scalar.activation(e0[:], sc01[:], AF.Exp, scale=0.125)
                    nc.tensor.matmul(sc23[:, 0:512], kT[p][0:HD, ss],
                                     qT[p][0:HD, t0 + 512:t0 + 1024],
                                     start=True, stop=True)
                    nc.tensor.matmul(sc23[:, 512:1024], kT[p][HD:P, ss],
                                     qT[p][HD:P, t0 + 512:t0 + 1024],
                                     start=True, stop=True)
                    nc.vector.tensor_scalar(e1[:], sc23[:], EA, EC,
                                            op0=ALU.mult, op1=ALU.add)
                    # ctx + denominators deferred ONE iteration
                    if prev is not None:
                        sp1, p1e0, p1e1 = prev
                        emit_ctx(sp1, p1e0[:], p1e1[:].bitcast(BF16),
                                 sp1 == 0, sp1 == ST - 1)
                        emit_dn(p1e0[:], p1e1[:].bitcast(BF16),
                                sp1 == 0, sp1 == ST - 1)
                    prev2 = prev
                    prev = (s, e0, e1)
                # drain s = ST-1: denominators first so the dn evict +
                # reciprocal chain starts as early as possible
                sp1, p1e0, p1e1 = prev
                emit_dn(p1e0[:], p1e1[:].bitcast(BF16),
                        sp1 == 0, sp1 == ST - 1)
                emit_ctx(sp1, p1e0[:], p1e1[:].bitcast(BF16),
                         sp1 == 0, sp1 == ST - 1)

                # dn evict FIRST on ACT (it gates the last block's
                # normalize chain), then the ctx evict.
                dnsb = nrmpool.tile([P, 512], F32, name="dnsb", tag="dnsb")
                if last:
                    nc.scalar.copy(dnsb[:], dn[:])
                    stg = acts.tile([P, 1024], F32, name="stgl",
                                    tag="stglast")
                    # split halves across ACT+DVE: the out-proj pool
                    # barrier waits on this evict, so halving its
                    # latency starts the tail tiles earlier
                    nc.scalar.copy(stg[:, 0:512], ctxAB[:, 0:512])
                    nc.vector.tensor_copy(stg[:, 512:1024],
                                          ctxAB[:, 512:1024])
                else:
                    nc.scalar.copy(dnsb[:], dn[:])
                    stg = stgpool.tile([P, 1024], F32, name="stg",
                                       tag="stg")
                    nc.scalar.copy(stg[:], ctxAB[:])

                # compact the 4 meaningful dn rows {0,32,64,96} into a
                # [128,16] tile (partition-major), then one fast DVE
                # reciprocal instruction.  rc partition p holds recip for
                # t = 16p.. (A in partitions 0-63, B in 64-127).
                dnc = nrmpool.tile([P, 16], F32, name="dnc", tag="dnc")
                nc.sync.dma_start(dnc[0:32, :], dnsb[0:1, :])
                nc.scalar.dma_start(dnc[32:64, :], dnsb[32:33, :])
                nc.sync.dma_start(dnc[64:96, :], dnsb[64:65, :])
                nc.scalar.dma_start(dnc[96:128, :], dnsb[96:97, :])
                rc = nrmpool.tile([P, 16], F32, name="rc", tag="rc")
                nc.vector.reciprocal_approx_fast(rc[:], dnc[:])
                if dbg and bi == 0:
                    nc.sync.dma_start(dbg["dn"][:, :], dnsb[:])
                    nc.sync.dma_start(dbg["rc"][:, :], rc[:])

                if not last:
                    # broadcast recip rows to a [128,1024] rb via log2
                    # DMA chains on the (idle) sync queue, then normalize
                    # on GpSimd - all off the PE critical path.
                    rb = rbpool.tile([P, 1024], F32, name="rb", tag="rb")
                    nc.sync.dma_start(rb[0:1, :], rc[0:HD, :])
                    nc.scalar.dma_start(rb[HD:HD + 1, :], rc[HD:P, :])
                    w = 1
                    while w < HD:
                        nc.sync.dma_start(rb[w:2 * w, :], rb[0:w, :])
                        nc.scalar.dma_start(rb[HD + w:HD + 2 * w, :],
                                            rb[HD:HD + w, :])
                        w *= 2
                    nc.gpsimd.tensor_tensor(out=ctxT[p][th][:], in0=stg[:],
                                            in1=rb[:], op=ALU.mult)
                    if dbg and bi == 0:
                        nc.scalar.dma_start(dbg["ctxT"][:, :],
                                            ctxT[p][th][:])
                else:
                    # tail: cast the recip rows to bf16 and land them in
                    # flt rows 0 / 64.  The K=1 PE broadcast + final DVE
                    # normalize are deferred into the out-proj section so
                    # the th=0 out-proj tiles run first (they only need
                    # blocks 0-1); the recip chain completes underneath
                    # them with no PE idle.
                    nc.vector.tensor_copy(rcb_l[:], rc[:])
                    nc.sync.dma_start(flt_l[0:1, :], rcb_l[0:HD, :])
                    nc.scalar.dma_start(flt_l[HD:HD + 1, :], rcb_l[HD:P, :])
                    stg_l = stg
        # ---- output projection -----------------------------------------
        with tc.tile_pool(name="popsum", bufs=3, space="PSUM") as popsum, \
             tc.tile_pool(name="rbpsum", bufs=1, space="PSUM") as rbpsum, \
             tc.tile_pool(name="opool", bufs=3) as opool:

            def emit_outproj(trange):
                for t in trange:
                    th_, tt_ = divmod(t, TT // 2)
                    ts_ = slice(tt_ * P, (tt_ + 1) * P)
                    po = popsum.tile([P, D], F32, name="po", tag="po")
                    for g in range(2):
                        for n in range(2):
                            ns = slice(n * 512, (n + 1) * 512)
                            nc.tensor.matmul(po[:, ns],
                                             ctxT[g][th_][:, ts_],
                                             wo_sb[g][:, ns],
                                             start=(g == 0), stop=(g == 1))
                    ost = opool.tile([P, D], BF16, name="ost", tag="ost")
                    if t % 2 == 0:
                        nc.vector.tensor_copy(ost[:], po[:])
                    else:
                        nc.scalar.copy(ost[:], po[:])
                    eng = nc.sync if t % 2 == 0 else nc.scalar
                    eng.dma_start(out_d[t * P:(t + 1) * P, :], ost[:])


            emit_outproj(range(1, 5))
            rbp = rbpsum.tile([P, 1024], F32, name="rbp", tag="rbp")
            nc.tensor.matmul(rbp[0:HD, 0:512], onesrow[0:1, :],
                             flt_l[0:1, 0:512], start=True, stop=True)
            nc.tensor.matmul(rbp[0:HD, 512:1024], onesrow[0:1, :],
                             flt_l[0:1, 512:1024], start=True, stop=True)
            nc.tensor.matmul(rbp[HD:P, 0:512], onesrow[HD:HD + 1, :],
                             flt_l[HD:HD + 1, 0:512], start=True, stop=True)
            nc.tensor.matmul(rbp[HD:P, 512:1024], onesrow[HD:HD + 1, :],
                             flt_l[HD:HD + 1, 512:1024],
                             start=True, stop=True)
            nc.vector.tensor_tensor(out=ctxT[1][1][:], in0=stg_l[:],
                                    in1=rbp[:], op=ALU.mult)
            emit_outproj(range(5, TT // 2))
            emit_outproj(range(TT // 2, TT))


def make_in_maps(query, key, value, Wq, bq, Wk, bk, Wv, bv, Wo, bo):
    """Shard the full inputs into the 8 per-core input dicts."""
    query, key, value, Wq, bq, Wk, bk, Wv, bv, Wo, bo = [
        np.asarray(a, dtype=np.float32)
        for a in (query, key, value, Wq, bq, Wk, bk, Wv, bv, Wo, bo)]

    def bf(a):
        return np.ascontiguousarray(a).astype(BF16_NP)

    in_maps = []
    for c in range(N_CORES):
        b, g = divmod(c, 4)
        sl = slice(g * OUTL, (g + 1) * OUTL)
        in_maps.append({
            "xq": bf(query[b].T),
            "xk": bf(key[b].T),
            "xv": bf(value[b].T),
            "wq": bf(Wq[sl, :].T),
            "wk": bf(Wk[sl, :].T),
            "wv": bf(Wv[sl, :].T),
            "wo": bf(Wo[:, sl].T),
            "bq": np.ascontiguousarray(bq[sl].reshape(2, P).T),
            "bk": np.ascontiguousarray(bk[sl].reshape(2, P).T),
        })
    return in_maps


def gather_out(results, Wo, bo, bv):
    """Sum the per-core bf16 partials and add the host-side bias terms."""
    Wo = np.asarray(Wo, np.float32)
    bo = np.asarray(bo, np.float32)
    bv = np.asarray(bv, np.float32)
    host_bias = bo + bv @ Wo.T
    out = np.empty((2, T, D), dtype=np.float32)
    for b in range(2):
        acc = results[4 * b]["out"].astype(np.float32)
        for g in range(1, 4):
            acc = acc + results[4 * b + g]["out"].astype(np.float32)
        out[b] = acc + host_bias
    return out


_NC_CACHE = None


def _get_nc():
    global _NC_CACHE
    if _NC_CACHE is None:
        _NC_CACHE = build_program()
    return _NC_CACHE


def kernel(query, key, value, Wq, bq, Wk, bk, Wv, bv, Wo, bo):
    nc = _get_nc()
    in_maps = make_in_maps(query, key, value, Wq, bq, Wk, bk, Wv, bv, Wo, bo)
    res = run_bass_kernel_spmd(nc, in_maps, list(range(N_CORES))).results
    return gather_out(res, Wo, bo, bv)


# revision 40
# speedup vs baseline: 1.1356x; 1.1356x over previous
"""Bass/Tile TRN2 kernel for nn_MultiHeadAttention_9277129359942.

B=2, T=S=2048, D=1024, H=16 heads, head_dim=64, fp32 I/O.

Sharding (8 cores): data-parallel over batch (2) x tensor-parallel over
head groups (4 heads / core, 256 out dims).  Each core computes the
attention for its 4 heads and a partial output projection; the host sums
the 4 bf16 partials per batch and adds the (linear) bo and bv terms
exactly: out = sum_g partial_g + bo + bv @ Wo.T.

v6 design (~221-228us measured vs v3 baseline ~315us; rel err 1.396e-2):
  - ctx matmuls col-tile-packed: head A -> psum partitions 0-63
    (col groups 0,1), head B -> 64-127 (groups 2,3), concurrent on the
    PE array.  The ones-column denominator trick is replaced by four
    M=1 denominator matmuls per iteration, col-tiled to psum partitions
    0/32/64/96 of one shared bank - all four run concurrently, so the
    denominators cost ~1 extra matmul-stream instead of widening ctx
    to M=65 (which would forfeit the 2x col packing).
  - exp split by t-chunk instead of by head: ACT exact exp on
    [scoresA-c0 | scoresB-c0] (one [128,1024] instr), DVE fast-exp
    (Schraudolph int16) on [A-c1 | B-c1].  Steady-state iteration is
    ~1.74us, bound by the DVE fast-exp (fp32-psum-read limited,
    ~1.22us) -> scores WAR cycle; psum (8 banks: sc 4 + ctx 2 + dn 1
    + scratch 1) cannot double-buffer the score tiles.
  - the ACT engine doubles as the scalar HWDGE queue: ACT compute ops
    emitted after bulk DMAs serialize behind the whole outstanding
    input stream.  Hence the exp-table preload is emitted before the
    x/w DMA block and ALL projection-phase evictions run on the DVE
    (pass 2 of the q/k projection is group-outer with inline evicts;
    the v projection is k-outer chasing the xv DMAs, with the psum
    `start` flag only on the even-s matmul of each shared bank - the
    reset clears the full bank row of every written partition).
  - blocks stream back-to-back on the PE; per-block normalize:
    denominators drain first, dn evicts, 4 compact DMAs -> [128,16] ->
    one reciprocal_approx_fast, then a log2 DMA broadcast chain +
    GpSimd multiply (blocks 0-2, fully off the critical path).  The
    last block casts the recip to bf16 and defers its broadcast (two
    pairs of concurrent K=1 PE matmuls into popsum) until after the
    th=0 out-proj tiles, so the tail has no DMA-latency stall; an
    out-proj bridge tile at the last block boundary keeps HAM (the PE
    clock throttle) at full clock through the transition.
  - biases are host-pre-transposed to [128,2] (a rearranged 256-row
    DRAM DMA was scheduled at the queue tail and stalled every
    consumer ~40us).
"""

import os
import sys

import numpy as np

for _p in ("/opt/trn_rl_repo",):
    if os.path.isdir(_p) and _p not in sys.path:
        sys.path.append(_p)

import ml_dtypes

import concourse.bass as bass
import concourse.mybir as mybir
import concourse.tile as tile
from concourse import bacc
from concourse.bass_utils import run_bass_kernel_spmd

F32 = mybir.dt.float32
BF16 = mybir.dt.bfloat16
I16 = mybir.dt.int16
AF = mybir.ActivationFunctionType
ALU = mybir.AluOpType
BF16_NP = ml_dtypes.bfloat16

D = 1024          # model dim
T = 2048          # query length
S = 2048          # key length
P = 128           # partitions
KT = D // P       # 8 contraction tiles
TT = T // P       # 16 row tiles
ST = S // P       # 16 key tiles
HL = 4            # local heads per core
HD = 64           # head dim
OUTL = HL * HD    # 256 local out dims
N_CORES = 8

# fast-exp constants: exp(x*0.125) ~= bf16(bitcast(int16(x*EA + EC)))
EA = float(0.125 * 128.0 / np.log(2.0))
EC = float(127 * 128 - 7.5)
I32 = mybir.dt.int32
RMAGIC = 0x7EF311C3   # int-trick reciprocal seed constant


def build_program():
    """Build + compile the SPMD program (same on all 8 cores)."""
    nc = bacc.Bacc(
        "TRN2", target_bir_lowering=False, debug=False, enable_asserts=True,
        num_devices=N_CORES,
    )

    xq_d = nc.dram_tensor("xq", [D, T], BF16, kind="ExternalInput")
    xk_d = nc.dram_tensor("xk", [D, S], BF16, kind="ExternalInput")
    xv_d = nc.dram_tensor("xv", [D, S], BF16, kind="ExternalInput")
    wq_d = nc.dram_tensor("wq", [D, OUTL], BF16, kind="ExternalInput")
    wk_d = nc.dram_tensor("wk", [D, OUTL], BF16, kind="ExternalInput")
    wv_d = nc.dram_tensor("wv", [D, OUTL], BF16, kind="ExternalInput")
    wo_d = nc.dram_tensor("wo", [OUTL, D], BF16, kind="ExternalInput")
    bq_d = nc.dram_tensor("bq", [P, 2], F32, kind="ExternalInput")
    bk_d = nc.dram_tensor("bk", [P, 2], F32, kind="ExternalInput")
    out_d = nc.dram_tensor("out", [T, D], BF16, kind="ExternalOutput")
    wsink_d = nc.dram_tensor("warm_sink", [1, 8], F32, kind="ExternalOutput")
    dbg = {}
    if os.environ.get("BASS_MHA_DEBUG"):
        dbg["v"] = nc.dram_tensor("dbg_v", [P, ST * HL * HD], BF16,
                                  kind="ExternalOutput")
        dbg["dn"] = nc.dram_tensor("dbg_dn", [P, 512], F32,
                                   kind="ExternalOutput")
        dbg["rc"] = nc.dram_tensor("dbg_rc", [P, 16], F32,
                                   kind="ExternalOutput")
        dbg["ctxT"] = nc.dram_tensor("dbg_ctxT", [P, 1024], BF16,
                                     kind="ExternalOutput")
        dbg["qT"] = nc.dram_tensor("dbg_qT", [P, T], BF16,
                                   kind="ExternalOutput")
        dbg["kT"] = nc.dram_tensor("dbg_kT", [P, S], BF16,
                                   kind="ExternalOutput")

    with tile.TileContext(nc) as tc:
        _build(nc, tc, xq_d, xk_d, xv_d, wq_d, wk_d, wv_d, wo_d,
               bq_d, bk_d, out_d, wsink_d, dbg)
    nc.compile()
    return nc


def _build(nc, tc, xq_d, xk_d, xv_d, wq_d, wk_d, wv_d, wo_d,
           bq_d, bk_d, out_d, wsink_d, dbg={}):
    from contextlib import ExitStack

    stack = ExitStack()
    with stack:
        consts = stack.enter_context(tc.tile_pool(name="consts", bufs=1))
        wpool = stack.enter_context(tc.tile_pool(name="wpool", bufs=1))
        acts = stack.enter_context(tc.tile_pool(name="acts", bufs=1))

        bq_sb = consts.tile([P, 2], F32, name="bq", tag="bq")
        bk_sb = consts.tile([P, 2], F32, name="bk", tag="bk")
        wsnk = consts.tile([1, 8], F32, name="wsnk", tag="wsnk")
        ones_sb = consts.tile([P, 1], BF16, name="ones", tag="ones")
        onesrow = consts.tile([P, HD], BF16, name="onesrow", tag="onesrow")

        wq_sb = [wpool.tile([P, OUTL], BF16, name=f"wq{k}", tag=f"wq{k}")
                 for k in range(KT)]
        wk_sb = [wpool.tile([P, OUTL], BF16, name=f"wk{k}", tag=f"wk{k}")
                 for k in range(KT)]
        wv_sb = [wpool.tile([P, OUTL], BF16, name=f"wv{k}", tag=f"wv{k}")
                 for k in range(KT)]
        wo_sb = [wpool.tile([P, D], BF16, name=f"wo{k}", tag=f"wo{k}")
                 for k in range(2)]

        qT = [acts.tile([P, T], BF16, name=f"qT{m}", tag=f"qT{m}")
              for m in range(2)]
        kT = [acts.tile([P, S], BF16, name=f"kT{m}", tag=f"kT{m}")
              for m in range(2)]
        v_sb = acts.tile([P, ST * HL * HD], BF16, name="vsb", tag="vsb")
        ctxT = [[acts.tile([P, 1024], BF16, name=f"ctxT{g}{th}",
                           tag=f"ctxT{g}{th}") for th in range(2)]
                for g in range(2)]

        xpool_cm = tc.tile_pool(name="xpool", bufs=1)
        xpool = xpool_cm.__enter__()
        xq_sb = [xpool.tile([P, T], BF16, name=f"xq{k}", tag=f"xq{k}")
                 for k in range(KT)]
        xk_sb = [xpool.tile([P, S], BF16, name=f"xk{k}", tag=f"xk{k}")
                 for k in range(KT)]
        xv_sb = [xpool.tile([P, S], BF16, name=f"xv{k}", tag=f"xv{k}")
                 for k in range(KT)]

        # Full 128-partition DMAs (all 16 SDMA engines per transfer),
        # alternating sync/scalar queues; q/k first, v later, wo last.
        nc.sync.dma_start(wq_sb[0][:], wq_d[0:P, :])
        nc.scalar.dma_start(wk_sb[0][:], wk_d[0:P, :])
        nc.sync.dma_start(bq_sb[:], bq_d[:, :])
        nc.scalar.dma_start(bk_sb[:], bk_d[:, :])
        nc.vector.memset(ones_sb[:], 1.0)
        nc.vector.memset(onesrow[:], 1.0)
        # ACT exp-table preload, emitted before the bulk DMA stream so the
        # in-order ACT queue isn't pinned behind the full input stream
        nc.scalar.activation(wsnk[0:1, 0:2], bq_sb[0:1, 0:2], AF.Exp)
        nc.sync.dma_start(xq_sb[0][:], xq_d[0:P, :])
        nc.scalar.dma_start(xk_sb[0][:], xk_d[0:P, :])
        for k in range(1, KT):
            nc.sync.dma_start(wq_sb[k][:], wq_d[k * P:(k + 1) * P, :])
            nc.scalar.dma_start(wk_sb[k][:], wk_d[k * P:(k + 1) * P, :])
            nc.sync.dma_start(xq_sb[k][:], xq_d[k * P:(k + 1) * P, :])
            nc.scalar.dma_start(xk_sb[k][:], xk_d[k * P:(k + 1) * P, :])
        for k in range(KT):
            eng = nc.sync if k % 2 == 0 else nc.scalar
            eng.dma_start(wv_sb[k][:], wv_d[k * P:(k + 1) * P, :])
            eng2 = nc.scalar if k % 2 == 0 else nc.sync
            eng2.dma_start(xv_sb[k][:], xv_d[k * P:(k + 1) * P, :])
        for k in range(2):
            eng = nc.sync if k % 2 == 0 else nc.scalar
            eng.dma_start(wo_sb[k][:], wo_d[k * P:(k + 1) * P, :])

        # ---- q/k projections: k-outer (chases the x DMAs) over two
        # t-half passes; all four (m, q/k) psum groups live at once -----
        with tc.tile_pool(name="qkpsum", bufs=1, space="PSUM") as qkpsum:
            # PE warmup burst (HAM un-throttle) during the DMA head
            warm = qkpsum.tile([P, 1024], F32, name="pq00", tag="pq0")
            for w in range(16):
                nc.tensor.matmul(warm[:, 0:OUTL], wq_sb[0][:, 0:P],
                                 wq_sb[0][:], start=(w == 0), stop=(w == 15))
            nc.vector.tensor_copy(wsnk[0:1, 2:4], warm[0:1, 0:2])
            nc.sync.dma_start(wsink_d[:, :], wsnk[:])

            groups = [(m, w_sb, x_sb, b_sb, o_sb)
                      for m in range(2)
                      for w_sb, x_sb, b_sb, o_sb in
                      ((wq_sb, xq_sb, bq_sb, qT),
                       (wk_sb, xk_sb, bk_sb, kT))]
            for tg in range(2):
                t_lo = tg * 1024
                ps_g = [qkpsum.tile([P, 1024], F32, name=f"pq{gi}{tg}",
                                    tag=f"pq{gi}")
                        for gi in range(4)]
                if tg == 0:
                    # k-outer: chases the streaming x DMAs
                    for k in range(KT):
                        for gi, (m, w_sb, x_sb, b_sb, o_sb) in \
                                enumerate(groups):
                            for c in range(2):
                                cs = slice(c * 512, (c + 1) * 512)
                                xs = slice(t_lo + c * 512,
                                           t_lo + (c + 1) * 512)
                                nc.tensor.matmul(
                                    ps_g[gi][:, cs],
                                    w_sb[k][:, m * P:(m + 1) * P],
                                    x_sb[k][:, xs],
                                    start=(k == 0), stop=(k == KT - 1))
                else:
                    # group-outer: inputs resident; evict each group as it
                    # completes so the next phase's psum frees early
                    for gi, (m, w_sb, x_sb, b_sb, o_sb) in \
                            enumerate(groups):
                        for k in range(KT):
                            for c in range(2):
                                cs = slice(c * 512, (c + 1) * 512)
                                xs = slice(t_lo + c * 512,
                                           t_lo + (c + 1) * 512)
                                nc.tensor.matmul(
                                    ps_g[gi][:, cs],
                                    w_sb[k][:, m * P:(m + 1) * P],
                                    x_sb[k][:, xs],
                                    start=(k == 0), stop=(k == KT - 1))
                        eng = nc.vector if gi % 2 == 0 else nc.scalar
                        if gi % 2 == 0:
                            nc.vector.tensor_scalar(
                                o_sb[m][:, t_lo:t_lo + 1024], ps_g[gi][:],
                                b_sb[:, m:m + 1], None, op0=ALU.add)
                        else:
                            nc.scalar.activation(
                                o_sb[m][:, t_lo:t_lo + 1024], ps_g[gi][:],
                                AF.Identity, bias=b_sb[:, m:m + 1])
                if tg == 0:
                    for gi, (m, w_sb, x_sb, b_sb, o_sb) in \
                            enumerate(groups):
                        # DVE, not ACT: the ACT engine is also the scalar
                        # HWDGE queue; a compute op there serializes behind
                        # the outstanding input-DMA stream.
                        nc.vector.tensor_scalar(
                            o_sb[m][:, t_lo:t_lo + 1024], ps_g[gi][:],
                            b_sb[:, m:m + 1], None, op0=ALU.add)

        # ---- v projection (no bias: bv is applied on the host).  k-outer
        # so the matmuls chase the xv DMAs; all 16 s-accumulators live in
        # psum (8 banks, 2 s-outputs per [128,512] tile) -----------------
        with tc.tile_pool(name="vpsum", bufs=1, space="PSUM") as vpsum:
            pvs = [vpsum.tile([P, 2 * OUTL], F32, name=f"pv{j}", tag=f"pv{j}")
                   for j in range(ST // 2)]
            # psum `start` resets the FULL bank row of every partition the
            # matmul writes, so only the even-s matmul of each shared bank
            # may carry start=True; the odd-s region accumulates onto the
            # columns that reset already zeroed.
            for k in range(KT):
                for s in range(ST):
                    dst = pvs[s // 2][:, (s % 2) * OUTL:(s % 2 + 1) * OUTL]
                    nc.tensor.matmul(
                        dst, xv_sb[k][:, s * P:(s + 1) * P], wv_sb[k][:],
                        start=(k == 0 and s % 2 == 0), stop=(k == KT - 1),
                        skip_group_check=True)
                    if k == KT - 1 and s % 2 == 1:
                        j = s // 2
                        if j % 2 == 0:
                            nc.vector.tensor_copy(
                                v_sb[:, j * 2 * OUTL:(j + 1) * 2 * OUTL],
                                pvs[j][:])
                        else:
                            nc.scalar.copy(
                                v_sb[:, j * 2 * OUTL:(j + 1) * 2 * OUTL],
                                pvs[j][:])
        xpool_cm.__exit__(None, None, None)
        if dbg:
            nc.sync.dma_start(dbg["v"][:, :], v_sb[:])
            nc.sync.dma_start(dbg["qT"][:, :], qT[0][:])
            nc.sync.dma_start(dbg["kT"][:, :], kT[0][:])

        # ---- attention: 4 blocks of (head pair p, t-half th), streamed
        # back-to-back on the PE -----------------------------------------
        with tc.tile_pool(name="scpsum", bufs=1, space="PSUM") as scpsum, \
             tc.tile_pool(name="ctxpsum", bufs=1, space="PSUM") as ctxpsum, \
             tc.tile_pool(name="dnpsum", bufs=1, space="PSUM") as dnpsum, \
             tc.tile_pool(name="stgpool", bufs=2) as stgpool, \
             tc.tile_pool(name="nrmpool", bufs=2) as nrmpool, \
             tc.tile_pool(name="rbpool", bufs=2) as rbpool, \
             tc.tile_pool(name="epool", bufs=2) as epool:

            rcb_l = acts.tile([P, 16], BF16, name="rcbl", tag="rcbl")
            stg_l = None
            flt_l = acts.tile([P, 1024], BF16, name="fltl", tag="fltl")
            # block order: th=0 blocks first so the th=0 out-proj tiles
            # are unblocked long before the tail
            blocks = [(0, 0), (1, 0), (0, 1), (1, 1)]
            obr_t = [acts.tile([P, D], BF16, name=f"obr{t}", tag=f"obr{t}")
                     for t in range(TT // 2)]

            def emit_bridge_chunk(ci):
                t, n = divmod(ci, 2)
                ts_ = slice(t * P, (t + 1) * P)
                ns = slice(n * 512, (n + 1) * 512)
                bpo = dnpsum.tile([P, 512], F32, name="bpo", tag="bpo")
                for g in range(2):
                    nc.tensor.matmul(bpo[:], ctxT[g][0][:, ts_],
                                     wo_sb[g][:, ns],
                                     start=(g == 0), stop=(g == 1))
                if n == 0:
                    nc.scalar.copy(obr_t[t][:, ns], bpo[:])
                else:
                    nc.vector.tensor_copy(obr_t[t][:, ns], bpo[:])
                    eng = nc.sync if t % 2 == 0 else nc.scalar
                    eng.dma_start(out_d[t * P:(t + 1) * P, :], obr_t[t][:])

            for bi, (p, th) in enumerate(blocks):
                t0 = th * 1024
                last = bi == len(blocks) - 1
                hA = 2 * p
                hB = 2 * p + 1
                if last:
                    # boundary bridge: out-proj tile t=0 keeps the PE
                    # dense across this block boundary (HAM stays at
                    # full clock)
                    emit_bridge_chunk(0)
                    emit_bridge_chunk(1)
                ctxAB = ctxpsum.tile([P, 1024], F32, name="ctxAB",
                                     tag="ctxAB")
                dn = dnpsum.tile([P, 512], F32, name="dn", tag="dn")

                def vsl(h, s):
                    return slice(s * OUTL + h * HD, s * OUTL + (h + 1) * HD)

                prev = None
                prev2 = None
                sc23_t = None

                def emit_dn(pe0x, pe1bx, st_, so_):
                    nc.tensor.matmul(dn[0:1, :], ones_sb[:],
                                     pe0x[:, 0:512], start=st_, stop=so_,
                                     tile_position=(0, 0))
                    nc.tensor.matmul(dn[32:33, :], ones_sb[:],
                                     pe1bx[:, 0:512], start=st_, stop=so_,
                                     tile_position=(0, 32))
                    nc.tensor.matmul(dn[64:65, :], ones_sb[:],
                                     pe0x[:, 512:1024], start=st_, stop=so_,
                                     tile_position=(0, 64))
                    nc.tensor.matmul(dn[96:97, :], ones_sb[:],
                                     pe1bx[:, 512:1024],
                                     start=st_, stop=so_,
                                     tile_position=(0, 96))

                def emit_ctx(spx, pe0x, pe1bx, st_, so_):
                    vA = v_sb[:, vsl(hA, spx)]
                    vB = v_sb[:, vsl(hB, spx)]
                    nc.tensor.matmul(ctxAB[0:HD, 0:512], vA,
                                     pe0x[:, 0:512], start=st_, stop=so_)
                    nc.tensor.matmul(ctxAB[HD:P, 0:512], vB,
                                     pe0x[:, 512:1024], start=st_, stop=so_)
                    nc.tensor.matmul(ctxAB[0:HD, 512:1024], vA,
                                     pe1bx[:, 0:512], start=st_, stop=so_)
                    nc.tensor.matmul(ctxAB[HD:P, 512:1024], vB,
                                     pe1bx[:, 512:1024],
                                     start=st_, stop=so_)

                for s in range(ST):
                    ss = slice(s * P, (s + 1) * P)
                    sc01 = scpsum.tile([P, 1024], F32, name="sc01",
                                       tag="sc01")
                    sc23 = scpsum.tile([P, 1024], F32, name="sc23",
                                       tag="sc23")
                    sc23_t = sc23
                    nc.tensor.matmul(sc01[:, 0:512], kT[p][0:HD, ss],
                                     qT[p][0:HD, t0:t0 + 512],
                                     start=True, stop=True)
                    nc.tensor.matmul(sc01[:, 512:1024], kT[p][HD:P, ss],
                                     qT[p][HD:P, t0:t0 + 512],
                                     start=True, stop=True)
                    e0 = epool.tile([P, 1024], BF16, name="e0", tag="e0")
                    e1 = epool.tile([P, 1024], I16, name="e1", tag="e1")
                    nc.scalar.activation(e0[:], sc01[:], AF.Exp, scale=0.125)
                    nc.tensor.matmul(sc23[:, 0:512], kT[p][0:HD, ss],
                                     qT[p][0:HD, t0 + 512:t0 + 1024],
                                     start=True, stop=True)
                    nc.tensor.matmul(sc23[:, 512:1024], kT[p][HD:P, ss],
                                     qT[p][HD:P, t0 + 512:t0 + 1024],
                                     start=True, stop=True)
                    nc.vector.tensor_scalar(e1[:], sc23[:], EA, EC,
                                            op0=ALU.mult, op1=ALU.add)
                    # ctx + denominators deferred ONE iteration
                    if prev is not None:
                        sp1, p1e0, p1e1 = prev
                        emit_ctx(sp1, p1e0[:], p1e1[:].bitcast(BF16),
                                 sp1 == 0, sp1 == ST - 1)
                        emit_dn(p1e0[:], p1e1[:].bitcast(BF16),
                                sp1 == 0, sp1 == ST - 1)
                    prev2 = prev
                    prev = (s, e0, e1)
                # drain s = ST-1: denominators first so the dn evict +
                # reciprocal chain starts as early as possible
                sp1, p1e0, p1e1 = prev
                emit_dn(p1e0[:], p1e1[:].bitcast(BF16),
                        sp1 == 0, sp1 == ST - 1)
                emit_ctx(sp1, p1e0[:], p1e1[:].bitcast(BF16),
                         sp1 == 0, sp1 == ST - 1)

                # dn evict FIRST on ACT (it gates the last block's
                # normalize chain), then the ctx evict.
                dnsb = nrmpool.tile([P, 512], F32, name="dnsb", tag="dnsb")
                if last:
                    nc.scalar.copy(dnsb[:], dn[:])
                    stg = acts.tile([P, 1024], F32, name="stgl",
                                    tag="stglast")
                    # split halves across ACT+DVE: the out-proj pool
                    # barrier waits on this evict, so halving its
                    # latency starts the tail tiles earlier
                    nc.scalar.copy(stg[:, 0:512], ctxAB[:, 0:512])
                    nc.vector.tensor_copy(stg[:, 512:1024],
                                          ctxAB[:, 512:1024])
                else:
                    nc.scalar.copy(dnsb[:], dn[:])
                    stg = stgpool.tile([P, 1024], F32, name="stg",
                                       tag="stg")
                    nc.scalar.copy(stg[:], ctxAB[:])

                # compact the 4 meaningful dn rows {0,32,64,96} into a
                # [128,16] tile (partition-major), then one fast DVE
                # reciprocal instruction.  rc partition p holds recip for
                # t = 16p.. (A in partitions 0-63, B in 64-127).
                dnc = nrmpool.tile([P, 16], F32, name="dnc", tag="dnc")
                nc.sync.dma_start(dnc[0:32, :], dnsb[0:1, :])
                nc.scalar.dma_start(dnc[32:64, :], dnsb[32:33, :])
                nc.sync.dma_start(dnc[64:96, :], dnsb[64:65, :])
                nc.scalar.dma_start(dnc[96:128, :], dnsb[96:97, :])
                rc = nrmpool.tile([P, 16], F32, name="rc", tag="rc")
                nc.vector.reciprocal_approx_fast(rc[:], dnc[:])
                if dbg and bi == 0:
                    nc.sync.dma_start(dbg["dn"][:, :], dnsb[:])
                    nc.sync.dma_start(dbg["rc"][:, :], rc[:])

                if not last:
                    # broadcast recip rows to a [128,1024] rb via log2
                    # DMA chains on the (idle) sync queue, then normalize
                    # on GpSimd - all off the PE critical path.
                    rb = rbpool.tile([P, 1024], F32, name="rb", tag="rb")
                    nc.sync.dma_start(rb[0:1, :], rc[0:HD, :])
                    nc.scalar.dma_start(rb[HD:HD + 1, :], rc[HD:P, :])
                    w = 1
                    while w < HD:
                        nc.sync.dma_start(rb[w:2 * w, :], rb[0:w, :])
                        nc.scalar.dma_start(rb[HD + w:HD + 2 * w, :],
                                            rb[HD:HD + w, :])
                        w *= 2
                    nc.gpsimd.tensor_tensor(out=ctxT[p][th][:], in0=stg[:],
                                            in1=rb[:], op=ALU.mult)
                    if dbg and bi == 0:
                        nc.scalar.dma_start(dbg["ctxT"][:, :],
                                            ctxT[p][th][:])
                else:
                    # tail: cast the recip rows to bf16 and land them in
                    # flt rows 0 / 64.  The K=1 PE broadcast + final DVE
                    # normalize are deferred into the out-proj section so
                    # the th=0 out-proj tiles run first (they only need
                    # blocks 0-1); the recip chain completes underneath
                    # them with no PE idle.
                    nc.vector.tensor_copy(rcb_l[:], rc[:])
                    nc.sync.dma_start(flt_l[0:1, :], rcb_l[0:HD, :])
                    nc.scalar.dma_start(flt_l[HD:HD + 1, :], rcb_l[HD:P, :])
                    stg_l = stg
        # ---- output projection -----------------------------------------
        with tc.tile_pool(name="popsum", bufs=3, space="PSUM") as popsum, \
             tc.tile_pool(name="rbpsum", bufs=1, space="PSUM") as rbpsum, \
             tc.tile_pool(name="opool", bufs=3) as opool:

            def emit_outproj(trange):
                for t in trange:
                    th_, tt_ = divmod(t, TT // 2)
                    ts_ = slice(tt_ * P, (tt_ + 1) * P)
                    po = popsum.tile([P, D], F32, name="po", tag="po")
                    for g in range(2):
                        for n in range(2):
                            ns = slice(n * 512, (n + 1) * 512)
                            nc.tensor.matmul(po[:, ns],
                                             ctxT[g][th_][:, ts_],
                                             wo_sb[g][:, ns],
                                             start=(g == 0), stop=(g == 1))
                    ost = opool.tile([P, D], BF16, name="ost", tag="ost")
                    if t % 2 == 0:
                        nc.vector.tensor_copy(ost[:], po[:])
                    else:
                        nc.scalar.copy(ost[:], po[:])
                    eng = nc.sync if t % 2 == 0 else nc.scalar
                    eng.dma_start(out_d[t * P:(t + 1) * P, :], ost[:])


            emit_outproj(range(1, 5))
            rbp = rbpsum.tile([P, 1024], F32, name="rbp", tag="rbp")
            nc.tensor.matmul(rbp[0:HD, 0:512], onesrow[0:1, :],
                             flt_l[0:1, 0:512], start=True, stop=True)
            nc.tensor.matmul(rbp[0:HD, 512:1024], onesrow[0:1, :],
                             flt_l[0:1, 512:1024], start=True, stop=True)
            nc.tensor.matmul(rbp[HD:P, 0:512], onesrow[HD:HD + 1, :],
                             flt_l[HD:HD + 1, 0:512], start=True, stop=True)
            nc.tensor.matmul(rbp[HD:P, 512:1024], onesrow[HD:HD + 1, :],
                             flt_l[HD:HD + 1, 512:1024],
                             start=True, stop=True)
            nc.vector.tensor_tensor(out=ctxT[1][1][:], in0=stg_l[:],
                                    in1=rbp[:], op=ALU.mult)
            emit_outproj(range(5, TT // 2))
            emit_outproj(range(TT // 2, TT))


def make_in_maps(query, key, value, Wq, bq, Wk, bk, Wv, bv, Wo, bo):
    """Shard the full inputs into the 8 per-core input dicts."""
    query, key, value, Wq, bq, Wk, bk, Wv, bv, Wo, bo = [
        np.asarray(a, dtype=np.float32)
        for a in (query, key, value, Wq, bq, Wk, bk, Wv, bv, Wo, bo)]

    def bf(a):
        return np.ascontiguousarray(a).astype(BF16_NP)

    in_maps = []
    for c in range(N_CORES):
        b, g = divmod(c, 4)
        sl = slice(g * OUTL, (g + 1) * OUTL)
        in_maps.append({
            "xq": bf(query[b].T),
            "xk": bf(key[b].T),
            "xv": bf(value[b].T),
            "wq": bf(Wq[sl, :].T),
            "wk": bf(Wk[sl, :].T),
            "wv": bf(Wv[sl, :].T),
            "wo": bf(Wo[:, sl].T),
            "bq": np.ascontiguousarray(bq[sl].reshape(2, P).T),
            "bk": np.ascontiguousarray(bk[sl].reshape(2, P).T),
        })
    return in_maps


def gather_out(results, Wo, bo, bv):
    """Sum the per-core bf16 partials and add the host-side bias terms."""
    Wo = np.asarray(Wo, np.float32)
    bo = np.asarray(bo, np.float32)
    bv = np.asarray(bv, np.float32)
    host_bias = bo + bv @ Wo.T
    out = np.empty((2, T, D), dtype=np.float32)
    for b in range(2):
        acc = results[4 * b]["out"].astype(np.float32)
        for g in range(1, 4):
            acc = acc + results[4 * b + g]["out"].astype(np.float32)
        out[b] = acc + host_bias
    return out


_NC_CACHE = None


def _get_nc():
    global _NC_CACHE
    if _NC_CACHE is None:
        _NC_CACHE = build_program()
    return _NC_CACHE


def kernel(query, key, value, Wq, bq, Wk, bk, Wv, bv, Wo, bo):
    nc = _get_nc()
    in_maps = make_in_maps(query, key, value, Wq, bq, Wk, bk, Wv, bv, Wo, bo)
    res = run_bass_kernel_spmd(nc, in_maps, list(range(N_CORES))).results
    return gather_out(res, Wo, bo, bv)
